# revision 8
# baseline (speedup 1.0000x reference)
"""Bipartite GCN (LightGCN-style) message passing on 8 TRN2 NeuronCores.

Strategy (destination-sharded, windowed matmul segment-sum):
  - Factored symmetric normalization: new_u = Du^-1/2 A Ds^-1/2 s, so tables
    are pre-scaled by r = deg^-1/2 once per layer and segment sums are plain.
  - Edges sorted by destination; destinations sharded across 8 cores (each
    core owns a contiguous slice of user rows and spot rows and receives every
    edge pointing into its slice -> its degree counts and sums are complete,
    no all-reduce; only the scaled gather tables are AllGather-replicated
    between layers).
  - Per destination window (<=128 dest rows), edges are processed in chunks of
    128: gather source rows (dma_gather, fp16 rows padded to 256B), build a
    one-hot [128 edges, 128 dests] on DVE (fp16 4x mode), accumulate
    psum[64 feats, window] += gathered.T @ onehot on PE, then transpose,
    scale by r (outputs) and r^2 (next gather table) and scatter-write.
  - Degrees are computed on device with the same one-hot machinery
    (lhsT=onehot, rhs=ones -> psum[128,1] counts).
  - Window edges are sorted by source for HBM page locality; spot-table
    indices exceed int16 so the spot table is gathered in two halves with a
    statically-sized lo/hi chunk-group layout per window.
"""

from contextlib import ExitStack

import numpy as np

import concourse.bass as bass
import concourse.bacc as bacc
import concourse.mybir as mybir
import concourse.tile as tile
from concourse.masks import make_identity
from concourse.bass_utils import run_bass_kernel_spmd

NCORES = 8
P = 128
D = 64
NUM_LAYERS = 3
OOB = 1 << 20
I16_MAX = 32768

f32 = mybir.dt.float32
fp16 = mybir.dt.float16
i32 = mybir.dt.int32
i16 = mybir.dt.int16


# ----------------------------------------------------------------------------
# host-side preprocessing
# ----------------------------------------------------------------------------

def _plan_direction(edge_dst, edge_src, n_dst, n_src, n_dst_core):
    """Plan one direction (dest-sharded windows/chunks/batches).

    Returns a config dict with per-core packed arrays:
      gidx   [core][128, B*64] int16   (wrapped+replicated gather indices)
      drel   [core][128, C] float32    (window-relative dest, -1 = pad)
      sidx   [core][W, 128] int32      (scatter rows, OOB = pad)
      and static sizes W (windows), K (chunks/window), B (gather batches).
    """
    halved = n_src > I16_MAX
    half = 0
    if halved:
        half = (n_src + 255) // 256 * 128  # split point, both halves < 32768
        assert half < I16_MAX and n_src - half <= half

    cores = []
    for c in range(NCORES):
        lo = c * n_dst_core
        hi = min((c + 1) * n_dst_core, n_dst)
        m = (edge_dst >= lo) & (edge_dst < hi)
        d = edge_dst[m] - lo
        s = edge_src[m]
        order = np.argsort(d, kind="stable")
        cores.append((d[order], s[order]))

    # greedy window cut: span <= 128. Within each window reorder edges by
    # (half, src): summation order is free, grouping gives static half-pure
    # batches and source-sorted gathers (HBM page locality).
    win_all = []   # per core: list of (base, lo_cnt, hi_cnt) per window
    gmax = 0
    for c in range(NCORES):
        d, s = cores[c]
        wins = []
        i = 0
        n = len(d)
        while i < n:
            base = d[i]
            j = np.searchsorted(d, base + 128, side="left")
            if halved:
                ord2 = np.lexsort((s[i:j], s[i:j] >= half))
            else:
                ord2 = np.argsort(s[i:j], kind="stable")
            d[i:j] = d[i:j][ord2]
            s[i:j] = s[i:j][ord2]
            if halved:
                lo_cnt = int(np.count_nonzero(s[i:j] < half))
                hi_cnt = (j - i) - lo_cnt
            else:
                lo_cnt, hi_cnt = j - i, 0
            wins.append((int(base), int(lo_cnt), int(hi_cnt)))
            gmax = max(gmax, lo_cnt, hi_cnt)
            i = j
        win_all.append(wins)

    # chunks per group, rounded to whole chunks; for halved tables also align
    # groups to 8-chunk gather batches so each batch is half-pure
    gch = (gmax + P - 1) // P
    if halved:
        gch = (gch + 7) // 8 * 8
        K = 2 * gch
    else:
        K = gch
    W = max(len(w) for w in win_all)
    C = W * K
    B = (C + 7) // 8  # 8-chunk gather batches

    out = dict(W=W, K=K, C=C, B=B, halved=halved, half=half, gch=gch,
               gidx=[], drel=[], sidx=[])
    for c in range(NCORES):
        d, s = cores[c]
        gi = np.zeros(B * 1024, np.int64)
        dr = np.full((P, C), -1.0, np.float32)
        si = np.full((W, P), OOB, np.int64)
        i = 0
        for w, (base, lo_cnt, hi_cnt) in enumerate(win_all[c]):
            span = 0
            for g, cnt in enumerate(([lo_cnt, hi_cnt] if halved else [lo_cnt])):
                ds = d[i:i + cnt]
                ss = s[i:i + cnt].copy()
                if halved and g == 1:
                    ss -= half
                c0 = (w * K + g * gch) * P
                gi[c0:c0 + cnt] = ss
                col = np.arange(cnt) // P + w * K + g * gch
                dr[np.arange(cnt) % P, col] = (ds - base).astype(np.float32)
                if cnt:
                    span = max(span, int(ds.max() - base) + 1)
                i += cnt
            si[w, :span] = base + np.arange(span)
        assert i == len(d)
        # wrap gather indices: idx i of batch b -> [i%16, b*64 + i//16]
        gi = gi.reshape(B, 64, 16).transpose(0, 2, 1)  # [B, 16, 64]
        gi = np.concatenate(list(gi), axis=1)  # [16, B*64]
        out["gidx"].append(gi.astype(np.int16))
        out["drel"].append(dr)
        out["sidx"].append(si.astype(np.int64))
    return out


def _host_prep(user_emb, spot_emb, edge_user, edge_spot):
    n_u, n_s = user_emb.shape[0], spot_emb.shape[0]
    nu_core = (n_u + NCORES - 1) // NCORES
    ns_core = (n_s + NCORES - 1) // NCORES
    cfg = dict(n_u=n_u, n_s=n_s, nu_core=nu_core, ns_core=ns_core,
               nup=nu_core * NCORES, nsp=ns_core * NCORES)
    # u-direction: dest = user, src = spot (gathers from spot table)
    cfg["u"] = _plan_direction(edge_user, edge_spot, n_u, cfg["nsp"], nu_core)
    # s-direction: dest = spot, src = user
    cfg["s"] = _plan_direction(edge_spot, edge_user, n_s, cfg["nup"], ns_core)

    u_pad = np.zeros((cfg["nup"], D), np.float32)
    u_pad[:n_u] = user_emb
    s_pad = np.zeros((cfg["nsp"], D), np.float32)
    s_pad[:n_s] = spot_emb

    iota = np.broadcast_to(np.arange(P, dtype=np.float16)[None, :], (P, P)).copy()
    ones = np.ones((P, 1), np.float16)

    in_maps = []
    for c in range(NCORES):
        m = dict(
            x0_u=u_pad[c * nu_core:(c + 1) * nu_core].copy(),
            x0_s=s_pad[c * ns_core:(c + 1) * ns_core].copy(),
            iota_c=iota, ones_c=ones,
        )
        for dn in ("u", "s"):
            dc = cfg[dn]
            m[f"gidx_{dn}"] = dc["gidx"][c]
            m[f"drel_{dn}"] = dc["drel"][c]
            sidx = dc["sidx"][c]
            m[f"sidx_{dn}"] = np.minimum(sidx, OOB).astype(np.int32).T.copy()
        in_maps.append(m)
    return cfg, in_maps


# ----------------------------------------------------------------------------
# bass program
# ----------------------------------------------------------------------------

def _build(cfg):
    nc = bacc.Bacc("TRN2", target_bir_lowering=False, debug=False,
                   num_devices=NCORES, num_swdge_queues=4)

    nu_core, ns_core = cfg["nu_core"], cfg["ns_core"]
    nup, nsp = cfg["nup"], cfg["nsp"]
    dcfg = {"u": cfg["u"], "s": cfg["s"]}
    vdst = {"u": nu_core, "s": ns_core}     # dest rows per core
    vsrc = {"u": nsp, "s": nup}             # global rows of gathered table
    other = {"u": "s", "s": "u"}

    inp = {}
    for dn in ("u", "s"):
        dc = dcfg[dn]
        inp[f"gidx_{dn}"] = nc.dram_tensor(f"gidx_{dn}", [16, dc["B"] * 64], i16,
                                           kind="ExternalInput")
        inp[f"drel_{dn}"] = nc.dram_tensor(f"drel_{dn}", [P, dc["C"]], f32,
                                           kind="ExternalInput")
        inp[f"sidx_{dn}"] = nc.dram_tensor(f"sidx_{dn}", [P, dc["W"]], i32,
                                           kind="ExternalInput")
    x0 = {"u": nc.dram_tensor("x0_u", [nu_core, D], f32, kind="ExternalInput"),
          "s": nc.dram_tensor("x0_s", [ns_core, D], f32, kind="ExternalInput")}
    iota_in = nc.dram_tensor("iota_c", [P, P], fp16, kind="ExternalInput")
    ones_in = nc.dram_tensor("ones_c", [P, 1], fp16, kind="ExternalInput")

    outs = {}
    for dn in ("u", "s"):
        for l in range(1, NUM_LAYERS + 1):
            outs[(dn, l)] = nc.dram_tensor(f"out_{dn}{l}", [vdst[dn], D], f32,
                                           kind="ExternalOutput")

    with tile.TileContext(nc) as tc, ExitStack() as ctx:
        sb = ctx.enter_context(tc.tile_pool(name="sb", bufs=1))
        gatp = ctx.enter_context(tc.tile_pool(name="gatp", bufs=8))
        ohp = ctx.enter_context(tc.tile_pool(name="ohp", bufs=4))
        psw = ctx.enter_context(tc.tile_pool(name="psw", bufs=4, space="PSUM"))
        pst = ctx.enter_context(tc.tile_pool(name="pst", bufs=2, space="PSUM"))
        epi = ctx.enter_context(tc.tile_pool(name="epi", bufs=4))
        dram = ctx.enter_context(tc.tile_pool(name="dram", bufs=1, space="DRAM"))

        # constants
        iota_t = sb.tile([P, P], fp16)
        nc.sync.dma_start(out=iota_t[:], in_=iota_in[:])
        ones_t = sb.tile([P, 1], fp16)
        nc.sync.dma_start(out=ones_t[:], in_=ones_in[:])
        ident = sb.tile([P, P], f32)
        make_identity(nc, ident[:])

        # static per-direction SBUF tables
        st = {}
        for dn in ("u", "s"):
            dc = dcfg[dn]
            t = {}
            t["drel"] = sb.tile([P, dc["C"]], f32, name=f"drel_t_{dn}")
            nc.sync.dma_start(out=t["drel"][:], in_=inp[f"drel_{dn}"][:])
            n = dc["B"] * 64
            t["gidx"] = sb.tile([P, n], i16, name=f"gidx_t_{dn}")
            step = 16384
            for o in range(0, n, step):
                w = min(step, n - o)
                nc.sync.dma_start(
                    out=t["gidx"][:, o:o + w],
                    in_=bass.AP(inp[f"gidx_{dn}"], o, [[0, 8], [n, 16], [1, w]]))
            t["sidx"] = sb.tile([P, dc["W"]], i32, name=f"sidx_t_{dn}")
            nc.sync.dma_start(out=t["sidx"][:], in_=inp[f"sidx_{dn}"][:])
            t["ru"] = sb.tile([P, dc["W"]], f32, name=f"ru_t_{dn}")
            t["ru2"] = sb.tile([P, dc["W"]], f32, name=f"ru2_t_{dn}")
            st[dn] = t

        # gather tables (padded fp16 [V, 128]) + AllGather staging, per layer
        zero128 = sb.tile([P, 2 * D], fp16)
        nc.vector.memset(zero128[:], 0.0)
        ag_in, ag_out = {}, {}
        for dn in ("u", "s"):
            v = vdst[dn]
            vg = vsrc[other[dn]]  # global rows for this direction's table
            for l in range(NUM_LAYERS):  # tilde_0 .. tilde_2
                t_in = dram.tile([v, 2 * D], fp16, name=f"ag_in_{dn}{l}")
                # zero-init: pad-edge gathers may read any row; NaN*0 = NaN
                for r in range(0, v, P):
                    h = min(P, v - r)
                    nc.sync.dma_start(out=t_in[r:r + h, :], in_=zero128[:h, :])
                ag_in[(dn, l)] = t_in
                ag_out[(dn, l)] = dram.tile([vg, 2 * D], fp16,
                                            addr_space="Shared",
                                            name=f"ag_out_{dn}{l}")

        def onehot_batch(dn, b):
            """one-hot tiles for chunks 8b..8b+7 -> fp16 [P, 8, P]"""
            dc = dcfg[dn]
            oh = ohp.tile([P, 8, P], fp16, tag="oh")
            for j in range(8):
                ch = 8 * b + j
                if ch >= dc["C"]:
                    break
                nc.vector.tensor_scalar(
                    out=oh[:, j, :], in0=iota_t[:],
                    scalar1=st[dn]["drel"][:, ch:ch + 1], scalar2=None,
                    op0=mybir.AluOpType.is_equal)
            return oh

        def deg_pass(dn):
            dc = dcfg[dn]
            for w in range(dc["W"]):
                pd = psw.tile([P, 1], f32, tag="pw", space="PSUM")
                for k in range(dc["K"]):
                    ch = w * dc["K"] + k
                    if ch % 8 == 0:
                        oh = onehot_batch(dn, ch // 8)
                    nc.tensor.matmul(out=pd[:], lhsT=oh[:, ch % 8, :],
                                     rhs=ones_t[:], start=(k == 0),
                                     stop=(k == dc["K"] - 1))
                dsafe = epi.tile([P, 1], f32, tag="dsafe")
                nc.vector.tensor_scalar(out=dsafe[:], in0=pd[:], scalar1=0.5,
                                        scalar2=None,
                                        op0=mybir.AluOpType.max)
                nc.vector.reciprocal(out=st[dn]["ru2"][:, w:w + 1], in_=dsafe[:])
                nc.scalar.activation(out=st[dn]["ru"][:, w:w + 1],
                                     in_=st[dn]["ru2"][:, w:w + 1],
                                     func=mybir.ActivationFunctionType.Sqrt)

        def tilde0(dn):
            dc = dcfg[dn]
            for w in range(dc["W"]):
                g0 = epi.tile([P, D], f32, tag="g0")
                nc.gpsimd.indirect_dma_start(
                    out=g0[:], out_offset=None, in_=x0[dn][:],
                    in_offset=bass.IndirectOffsetOnAxis(
                        ap=st[dn]["sidx"][:, w:w + 1], axis=0),
                    bounds_check=vdst[dn] - 1, oob_is_err=False)
                t16 = epi.tile([P, D], fp16, tag="t16")
                nc.vector.tensor_scalar(out=t16[:], in0=g0[:],
                                        scalar1=st[dn]["ru"][:, w:w + 1],
                                        scalar2=None,
                                        op0=mybir.AluOpType.mult)
                nc.gpsimd.indirect_dma_start(
                    out=ag_in[(dn, 0)][:], in_=t16[:], in_offset=None,
                    out_offset=bass.IndirectOffsetOnAxis(
                        ap=st[dn]["sidx"][:, w:w + 1], axis=0),
                    bounds_check=vdst[dn] - 1, oob_is_err=False)

        def main_pass(dn, l):
            """layer l in 1..3: dest-direction dn, gathers tilde_{l-1} of other"""
            dc = dcfg[dn]
            src_tab = ag_out[(other[dn], l - 1)]
            half_rows = dc["half"]
            for w in range(dc["W"]):
                pw = psw.tile([D, P], f32, tag="pw", space="PSUM")
                for k in range(dc["K"]):
                    ch = w * dc["K"] + k
                    if ch % 8 == 0:
                        b = ch // 8
                        gat = gatp.tile([P, 8, 2 * D], fp16, tag="gat")
                        if dc["halved"]:
                            in_hi = (k >= dc["gch"])
                            src_ap = (src_tab[half_rows:, :] if in_hi
                                      else src_tab[:half_rows, :])
                        else:
                            src_ap = src_tab[:]
                        nc.gpsimd.dma_gather(
                            out_ap=gat[:], in_ap=src_ap,
                            idxs_ap=st[dn]["gidx"][:, b * 64:(b + 1) * 64],
                            num_idxs=1024, num_idxs_reg=1024,
                            elem_size=2 * D, queue_num=b % 4)
                        oh = onehot_batch(dn, b)
                    nc.tensor.matmul(out=pw[:], lhsT=gat[:, ch % 8, 0:D],
                                     rhs=oh[:, ch % 8, :], start=(k == 0),
                                     stop=(k == dc["K"] - 1))
                # epilogue
                tw = epi.tile([D, P], f32, tag="tw")
                nc.vector.tensor_copy(out=tw[:], in_=pw[:])
                p2 = pst.tile([P, D], f32, tag="p2", space="PSUM")
                nc.tensor.transpose(out=p2[:], in_=tw[:], identity=ident[:D, :D])
                ot = epi.tile([P, D], f32, tag="ot")
                nc.vector.tensor_scalar(out=ot[:], in0=p2[:],
                                        scalar1=st[dn]["ru"][:, w:w + 1],
                                        scalar2=None, op0=mybir.AluOpType.mult)
                nc.gpsimd.indirect_dma_start(
                    out=outs[(dn, l)][:], in_=ot[:], in_offset=None,
                    out_offset=bass.IndirectOffsetOnAxis(
                        ap=st[dn]["sidx"][:, w:w + 1], axis=0),
                    bounds_check=vdst[dn] - 1, oob_is_err=False)
                if l < NUM_LAYERS:
                    t16 = epi.tile([P, D], fp16, tag="t16b")
                    nc.vector.tensor_scalar(out=t16[:], in0=p2[:],
                                            scalar1=st[dn]["ru2"][:, w:w + 1],
                                            scalar2=None,
                                            op0=mybir.AluOpType.mult)
                    nc.gpsimd.indirect_dma_start(
                        out=ag_in[(dn, l)][:], in_=t16[:], in_offset=None,
                        out_offset=bass.IndirectOffsetOnAxis(
                            ap=st[dn]["sidx"][:, w:w + 1], axis=0),
                        bounds_check=vdst[dn] - 1, oob_is_err=False)

        def allgather(dn, l):
            nc.gpsimd.collective_compute(
                "AllGather", mybir.AluOpType.bypass,
                replica_groups=[list(range(NCORES))],
                ins=[ag_in[(dn, l)][:].opt()],
                outs=[ag_out[(dn, l)][:].opt()])

        deg_pass("u")
        deg_pass("s")
        tilde0("u")
        tilde0("s")
        tc.strict_bb_all_engine_barrier()
        allgather("u", 0)
        allgather("s", 0)
        tc.strict_bb_all_engine_barrier()
        for l in range(1, NUM_LAYERS + 1):
            main_pass("u", l)
            main_pass("s", l)
            if l < NUM_LAYERS:
                tc.strict_bb_all_engine_barrier()
                allgather("u", l)
                allgather("s", l)
                tc.strict_bb_all_engine_barrier()

    nc.compile()
    return nc


# ----------------------------------------------------------------------------
# entry point
# ----------------------------------------------------------------------------

_CACHE = {}


def _get_compiled(user_emb, spot_emb, edge_user, edge_spot):
    cfg, in_maps = _host_prep(np.asarray(user_emb), np.asarray(spot_emb),
                              np.asarray(edge_user), np.asarray(edge_spot))
    key = (cfg["n_u"], cfg["n_s"], cfg["u"]["W"], cfg["u"]["K"],
           cfg["s"]["W"], cfg["s"]["K"])
    if key not in _CACHE:
        _CACHE[key] = _build(cfg)
    return cfg, in_maps, _CACHE[key]


def kernel(user_emb, spot_emb, edge_user, edge_spot):
    user_emb = np.asarray(user_emb, dtype=np.float32)
    spot_emb = np.asarray(spot_emb, dtype=np.float32)
    edge_user = np.asarray(edge_user, dtype=np.int64)
    edge_spot = np.asarray(edge_spot, dtype=np.int64)
    cfg, in_maps, nc = _get_compiled(user_emb, spot_emb, edge_user, edge_spot)

    res = run_bass_kernel_spmd(nc, in_maps, core_ids=list(range(NCORES)))
    results = res.results

    n_u, n_s = cfg["n_u"], cfg["n_s"]
    user_out = np.empty((n_u, NUM_LAYERS + 1, D), np.float32)
    spot_out = np.empty((n_s, NUM_LAYERS + 1, D), np.float32)
    user_out[:, 0] = user_emb
    spot_out[:, 0] = spot_emb
    for l in range(1, NUM_LAYERS + 1):
        uo = np.concatenate([results[c][f"out_u{l}"] for c in range(NCORES)],
                            axis=0)
        so = np.concatenate([results[c][f"out_s{l}"] for c in range(NCORES)],
                            axis=0)
        user_out[:, l] = uo[:n_u]
        spot_out[:, l] = so[:n_s]
    return (spot_out, user_out)
